# revision 35
# baseline (speedup 1.0000x reference)
"""Trainium2 Bass kernel for nn_AttentionReceiver.

Precision: the 21-step recurrence amplifies perturbations ~100x, so every
matmul runs in software bf16x2 (hi+lo bf16 split, fp32 PSUM accumulate):
products hi*hi + hi*lo + lo*hi(+lo*lo) give ~16-bit mantissa, final rel
error ~3e-4. Softmax/LSTM elementwise in fp32.

Memory: M in both orientations at 4B/elem exceeds SBUF, so each core
processes its 128 batches as two sequential waves of 64. W_ih/W_hh are
re-streamed from HBM per step (hidden under compute).

Layout: batch-on-partition for softmax/LSTM; feature-major (feature on
partition, batch on free) for matmul chains. Per-batch attention matvecs
keep M[b] stationary (128-col bf16 loads use fast-weight-load) and emit
full-partition feature-major columns (base partition 0: always legal).
Steps t=1..19 run in a hardware For_i loop; t=0 (h=0 shortcut: softmax of
the mask alone) and t=20 (attend-only) are peeled.
"""

import os
import sys

for _p in ("/opt/trn_rl_repo", "/opt/trn_rl_repo/concourse"):
    if _p not in sys.path:
        sys.path.insert(0, _p)

import numpy as np
import ml_dtypes

import concourse.bass as bass
import concourse.tile as tile
from concourse import bacc, mybir
from concourse.bass import ds
from concourse.masks import make_identity
from concourse import bass_utils

BF16 = ml_dtypes.bfloat16
F32 = np.float32

BS, L, E, H, T, V = 1024, 128, 256, 512, 20, 1000
NCORES = 8
BC = BS // NCORES    # 128 batches per core
NW = 2               # waves per core
BCW = BC // NW       # 64 batches per wave
KE = E // 128        # 2
KH = H // 128        # 4
KF = (E + H) // 128  # 6
G4 = 4 * H           # 2048

_CACHED_NC = {}


def build_nc(nloop=1, unroll=False):
    fp32 = mybir.dt.float32
    bf16 = mybir.dt.bfloat16

    nc = bacc.Bacc("TRN2", target_bir_lowering=False, debug=False)

    def din(name, shape, dt=bf16):
        return nc.dram_tensor(name, shape, dt, kind="ExternalInput").ap()

    d_me = [din(f"me_{p}", [NW, 128, BCW, KE, 128]) for p in ("hi", "lo")]
    d_ml = [din(f"ml_{p}", [NW, 128, BCW, E]) for p in ("hi", "lo")]
    d_x = [din(f"x_{p}", [NW, 128, KE, T, BCW]) for p in ("hi", "lo")]
    d_mask = din("maskb", [NW, BCW, L], fp32)
    d_sel = din("sel", [NW, BCW, T + 1], fp32)
    d_wq = [din(f"wq_{p}", [128, KH, E]) for p in ("hi", "lo")]
    d_wc = [din(f"wc_{p}", [128, KF, H]) for p in ("hi", "lo")]
    d_wih = [din(f"wih_{p}", [128, KE, G4]) for p in ("hi", "lo")]
    d_whh = [din(f"whh_{p}", [128, KH, G4]) for p in ("hi", "lo")]
    d_bg = din("bg", [BCW, G4], bf16)
    d_out = nc.dram_tensor("out", [NW, BCW, L], fp32, kind="ExternalOutput").ap()

    ADD = mybir.AluOpType.add
    SUB = mybir.AluOpType.subtract
    MULT = mybir.AluOpType.mult
    MAX = mybir.AluOpType.max
    AXX = mybir.AxisListType.X
    EXP = mybir.ActivationFunctionType.Exp
    SIG = mybir.ActivationFunctionType.Sigmoid
    TANH = mybir.ActivationFunctionType.Tanh
    ET = mybir.EngineType

    with tile.TileContext(nc) as tc:
        with (
            tc.tile_pool(name="big", bufs=1) as big,
            tc.tile_pool(name="work", bufs=2) as work,
            tc.tile_pool(name="lstm", bufs=1) as lsp,
            tc.tile_pool(name="xin", bufs=2) as xin,
            tc.tile_pool(name="wst", bufs=2) as wst,
            tc.tile_pool(name="ps_mix", bufs=2, space="PSUM") as ps_mix,
            tc.tile_pool(name="ps_mix2", bufs=2, space="PSUM") as ps_mix2,
            tc.tile_pool(name="ps_g", bufs=2, space="PSUM") as ps_gp,
        ):
            # ---------- resident tiles ----------
            sb_me = [big.tile([128, BCW, KE, 128], bf16, name=f"sb_me_{p}")
                     for p in ("hi", "lo")]
            sb_ml = [big.tile([128, BCW, E], bf16, name=f"sb_ml_{p}")
                     for p in ("hi", "lo")]
            sb_wq = [big.tile([128, KH, E], bf16, name=f"sb_wq_{p}")
                     for p in ("hi", "lo")]
            sb_wc = [big.tile([128, KF, H], bf16, name=f"sb_wc_{p}")
                     for p in ("hi", "lo")]
            sb_bg = big.tile([BCW, G4], bf16)
            sb_mask = big.tile([BCW, L], fp32)
            sb_sel = big.tile([BCW, T + 1], fp32)
            ident32 = big.tile([128, 128], fp32)
            identb = big.tile([128, 128], bf16)
            h_fm = [big.tile([128, KH, BCW], bf16, name=f"h_fm_{p}")
                    for p in ("hi", "lo")]
            c_bm = big.tile([BCW, H], fp32)
            out_sb = big.tile([BCW, L], fp32)

            for p in range(2):
                nc.sync.dma_start(out=sb_wq[p], in_=d_wq[p])
                nc.sync.dma_start(out=sb_wc[p], in_=d_wc[p])
            nc.sync.dma_start(out=sb_bg, in_=d_bg)
            make_identity(nc, ident32)
            make_identity(nc, identb)

            def mm(out, lhsT, rhs, start, stop):
                nc.tensor.matmul(out, lhsT=lhsT, rhs=rhs, start=start,
                                 stop=stop)

            def softmax_accum(sc_src, t_sel):
                """softmax over L + sel-weighted fp32 accumulation.
                sc_src: [BCW, L] fp32 AP (already mask-included).
                Returns w_f32 [BCW, L]."""
                mx = work.tile([BCW, 1], fp32, tag="mx")
                nc.vector.tensor_reduce(mx, sc_src, axis=AXX, op=MAX)
                nmx = work.tile([BCW, 1], fp32, tag="nmx")
                nc.vector.tensor_scalar_mul(out=nmx, in0=mx, scalar1=-1.0)
                w_exp = work.tile([BCW, L], fp32, tag="w_exp")
                sume = work.tile([BCW, 1], fp32, tag="sume")
                nc.scalar.activation(w_exp, sc_src, EXP, bias=nmx, scale=1.0,
                                     accum_out=sume)
                rsum = work.tile([BCW, 1], fp32, tag="rsum")
                nc.vector.reciprocal(rsum, sume)
                w_f32 = work.tile([BCW, L], fp32, tag="w_f32")
                nc.vector.tensor_tensor(
                    w_f32, w_exp, rsum[:, 0:1].to_broadcast((BCW, L)), MULT)
                rs2 = work.tile([BCW, 1], fp32, tag="rs2")
                nc.vector.tensor_tensor(rs2, rsum, t_sel, MULT)
                wtmp = work.tile([BCW, L], fp32, tag="wtmp")
                nc.vector.tensor_tensor(
                    wtmp, w_exp, rs2[:, 0:1].to_broadcast((BCW, L)), MULT)
                nc.vector.tensor_tensor(out_sb, out_sb, wtmp, ADD)
                return w_f32

            def attend_scores(t_sel):
                """pq + scores + softmax + accumulate. Returns w_f32."""
                # pq = Wq @ h   (feature-major [e, b], bf16x2)
                ps_pq = ps_mix.tile([128, KE, BCW], fp32, tag="ps_mix")
                for mt in range(KE):
                    for kt in range(KH):
                        lw_hi = sb_wq[0][:, kt, mt * 128:(mt + 1) * 128]
                        lw_lo = sb_wq[1][:, kt, mt * 128:(mt + 1) * 128]
                        o = ps_pq[:, mt, :]
                        mm(o, lw_hi, h_fm[0][:, kt, :], kt == 0, False)
                        mm(o, lw_hi, h_fm[1][:, kt, :], False, False)
                        mm(o, lw_lo, h_fm[0][:, kt, :], False, kt == KH - 1)
                # split pq -> [hi|lo] adjacent pairs
                pq_hl = work.tile([128, KE, BCW, 2], bf16, tag="pq_hl")
                nc.vector.tensor_copy(pq_hl[:, :, :, 0], ps_pq)
                nc.vector.tensor_tensor(pq_hl[:, :, :, 1], ps_pq,
                                        pq_hl[:, :, :, 0], SUB)
                # scores.T columns: stationary M[b] chunks, rhs = pq pair
                ps_sc = ps_mix2.tile([128, BCW, 2], fp32, tag="ps_mix2")
                for b in range(BCW):
                    o = ps_sc[:, b, :]
                    for ke in range(KE):
                        mm(o, sb_me[0][:, b, ke, :], pq_hl[:, ke, b, :],
                           ke == 0, False)
                        mm(o, sb_me[1][:, b, ke, :], pq_hl[:, ke, b, :],
                           False, ke == KE - 1)
                sc_pair = work.tile([128, BCW, 2], fp32, tag="sc_pair")
                nc.vector.tensor_copy(sc_pair, ps_sc)
                sc_fm = work.tile([128, BCW], fp32, tag="sc_fm")
                nc.vector.tensor_tensor(sc_fm, sc_pair[:, :, 0],
                                        sc_pair[:, :, 1], ADD)
                ps_scB = ps_mix2.tile([BCW, 128], fp32, tag="ps_mix2")
                nc.tensor.transpose(ps_scB, sc_fm, ident32)
                sc_sb = work.tile([BCW, L], fp32, tag="sc_sb")
                nc.vector.tensor_tensor(sc_sb, ps_scB, sb_mask, ADD)
                return softmax_accum(sc_sb, t_sel)

            def applied_att(w_f32):
                """w split/transpose, applied matvecs, att projection.
                Returns (att_hi, att_lo) feature-major [128, KH, BCW]."""
                w_hi = work.tile([BCW, L], bf16, tag="w_hi")
                nc.vector.tensor_copy(w_hi, w_f32)
                w_lo = work.tile([BCW, L], bf16, tag="w_lo")
                nc.vector.tensor_tensor(w_lo, w_f32, w_hi, SUB)
                w_fhl = work.tile([128, BCW, 2], bf16, tag="w_fhl")
                for p, wt in ((0, w_hi), (1, w_lo)):
                    ps_wT = ps_mix2.tile([128, BCW], bf16, tag="ps_mix2")
                    nc.tensor.transpose(ps_wT, wt, identb[0:BCW, 0:BCW])
                    nc.vector.tensor_copy(w_fhl[:, :, p], ps_wT)
                # applied.T columns: stationary M_l[b] chunks, rhs = w pair
                ps_ap = ps_mix.tile([128, KE, BCW, 2], fp32, tag="ps_mix")
                for b in range(BCW):
                    wpair = w_fhl[:, b, :]
                    for mt in range(KE):
                        o = ps_ap[:, mt, b, :]
                        mm(o, sb_ml[0][:, b, mt * 128:(mt + 1) * 128], wpair,
                           True, False)
                        mm(o, sb_ml[1][:, b, mt * 128:(mt + 1) * 128], wpair,
                           False, True)
                ap_pair = work.tile([128, KE, BCW, 2], fp32, tag="ap_pair")
                nc.vector.tensor_copy(ap_pair, ps_ap)
                ap_f32 = work.tile([128, KE, BCW], fp32, tag="ap_f32")
                nc.vector.tensor_tensor(ap_f32, ap_pair[:, :, :, 0],
                                        ap_pair[:, :, :, 1], ADD)
                ap_hi = work.tile([128, KE, BCW], bf16, tag="ap_hi")
                nc.vector.tensor_copy(ap_hi, ap_f32)
                ap_lo = work.tile([128, KE, BCW], bf16, tag="ap_lo")
                nc.vector.tensor_tensor(ap_lo, ap_f32, ap_hi, SUB)
                # att = Wc @ [applied; h]  (feature-major, bf16x2)
                ps_att = ps_mix.tile([128, KH, BCW], fp32, tag="ps_mix")
                for mt in range(KH):
                    o = ps_att[:, mt, :]
                    for kf in range(KF):
                        if kf < KE:
                            r_hi, r_lo = ap_hi[:, kf, :], ap_lo[:, kf, :]
                        else:
                            r_hi = h_fm[0][:, kf - KE, :]
                            r_lo = h_fm[1][:, kf - KE, :]
                        lw_hi = sb_wc[0][:, kf, mt * 128:(mt + 1) * 128]
                        lw_lo = sb_wc[1][:, kf, mt * 128:(mt + 1) * 128]
                        mm(o, lw_hi, r_hi, kf == 0, False)
                        mm(o, lw_hi, r_lo, False, False)
                        mm(o, lw_lo, r_hi, False, kf == KF - 1)
                att_hi = work.tile([128, KH, BCW], bf16, tag="att_hi")
                nc.vector.tensor_copy(att_hi, ps_att)
                att_lo = work.tile([128, KH, BCW], bf16, tag="att_lo")
                nc.vector.tensor_tensor(att_lo, ps_att, att_hi, SUB)
                return att_hi, att_lo

            def lstm_step(wave, t_dyn, att_hi, att_lo):
                """gates (streamed W) + LSTM elementwise + h split/transpose."""
                x_t = []
                for p in range(2):
                    xt = xin.tile([128, KE, 1, BCW], bf16, tag=f"x_{p}")
                    nc.sync.dma_start(out=xt, in_=d_x[p][wave][:, :, t_dyn, :])
                    x_t.append(xt)
                gsum = lsp.tile([BCW, G4], fp32, tag="gsum")
                NCH = 256
                for half in range(2):
                    ps_gh = ps_gp.tile([BCW, 1024], fp32, tag="ps_g")
                    for nch in range(1024 // NCH):
                        n0 = half * 1024 + nch * NCH
                        wb = []
                        for p in range(2):
                            wi = wst.tile([128, KE, NCH], bf16, tag=f"wih_{p}")
                            nc.sync.dma_start(out=wi,
                                              in_=d_wih[p][:, :, n0:n0 + NCH])
                            wh = wst.tile([128, KH, NCH], bf16, tag=f"whh_{p}")
                            nc.sync.dma_start(out=wh,
                                              in_=d_whh[p][:, :, n0:n0 + NCH])
                            wb.append((wi, wh))
                        o = ps_gh[:, nch * NCH:(nch + 1) * NCH]
                        for kt in range(KF):
                            if kt < KE:
                                a_hi = x_t[0][:, kt, 0, :]
                                a_lo = x_t[1][:, kt, 0, :]
                                r_hi = wb[0][0][:, kt, :]
                                r_lo = wb[1][0][:, kt, :]
                            else:
                                a_hi = att_hi[:, kt - KE, :]
                                a_lo = att_lo[:, kt - KE, :]
                                r_hi = wb[0][1][:, kt - KE, :]
                                r_lo = wb[1][1][:, kt - KE, :]
                            mm(o, a_hi, r_hi, kt == 0, False)
                            mm(o, a_hi, r_lo, False, False)
                            mm(o, a_lo, r_hi, False, kt == KF - 1)
                    nc.vector.tensor_tensor(
                        gsum[:, half * 1024:(half + 1) * 1024], ps_gh,
                        sb_bg[:, half * 1024:(half + 1) * 1024], ADD)
                # gates order [i, f, o, g]
                sig_if = lsp.tile([BCW, 2 * H], fp32, tag="sig_if")
                nc.scalar.activation(sig_if, gsum[:, 0:2 * H], SIG)
                sig_o = lsp.tile([BCW, H], fp32, tag="sig_o")
                nc.scalar.activation(sig_o, gsum[:, 2 * H:3 * H], SIG)
                tanh_g = lsp.tile([BCW, H], fp32, tag="tanh_g")
                nc.scalar.activation(tanh_g, gsum[:, 3 * H:4 * H], TANH)
                t1 = lsp.tile([BCW, H], fp32, tag="t1")
                nc.vector.tensor_tensor(t1, sig_if[:, 0:H], tanh_g, MULT)
                nc.vector.tensor_tensor(c_bm, c_bm, sig_if[:, H:2 * H], MULT)
                nc.vector.tensor_tensor(c_bm, c_bm, t1, ADD)
                tanh_c = lsp.tile([BCW, H], fp32, tag="tanh_c")
                nc.scalar.activation(tanh_c, c_bm, TANH)
                h_f32 = lsp.tile([BCW, H], fp32, tag="h_f32")
                nc.vector.tensor_tensor(h_f32, sig_o, tanh_c, MULT)
                h_hi = lsp.tile([BCW, H], bf16, tag="h_hi")
                nc.vector.tensor_copy(h_hi, h_f32)
                h_lo = lsp.tile([BCW, H], bf16, tag="h_lo")
                nc.vector.tensor_tensor(h_lo, h_f32, h_hi, SUB)
                for p, hb in ((0, h_hi), (1, h_lo)):
                    for kt in range(KH):
                        ps_h = ps_mix2.tile([128, BCW], bf16, tag="ps_mix2")
                        nc.tensor.transpose(
                            ps_h, hb[:, kt * 128:(kt + 1) * 128],
                            identb[0:BCW, 0:BCW])
                        nc.vector.tensor_copy(h_fm[p][:, kt, :], ps_h)

            def wave_body(wave):
                # M + per-wave small tensors
                for p in range(2):
                    for q in range(4):
                        sl = slice(q * 16, (q + 1) * 16)
                        nc.sync.dma_start(out=sb_ml[p][:, sl],
                                          in_=d_ml[p][wave][:, sl])
                    for q in range(4):
                        sl = slice(q * 16, (q + 1) * 16)
                        nc.sync.dma_start(out=sb_me[p][:, sl],
                                          in_=d_me[p][wave][:, sl])
                nc.sync.dma_start(out=sb_mask, in_=d_mask[wave])
                nc.sync.dma_start(out=sb_sel, in_=d_sel[wave])
                nc.vector.memset(h_fm[0], 0.0)
                nc.vector.memset(h_fm[1], 0.0)
                nc.vector.memset(c_bm, 0.0)
                nc.vector.memset(out_sb, 0.0)

                # t=0 peeled: h=0 -> scores = mask only
                w0 = softmax_accum(sb_mask, sb_sel[:, 0:1])
                a_hi, a_lo = applied_att(w0)
                lstm_step(wave, ds(0, 1), a_hi, a_lo)
                # t = 1..19
                def mid_step(t):
                    wf = attend_scores(sb_sel[:, ds(t, 1)])
                    a_hi, a_lo = applied_att(wf)
                    lstm_step(wave, ds(t, 1), a_hi, a_lo)

                if unroll:
                    for t in range(1, T):
                        mid_step(t)
                else:
                    with tc.For_i(1, T, 1,
                                  hint_engines=(ET.PE, ET.DVE, ET.Activation,
                                                ET.SP)) as t:
                        mid_step(t)
                # t=20 peeled: attend only
                attend_scores(sb_sel[:, T:T + 1])
                nc.sync.dma_start(out=d_out[wave], in_=out_sb)

            if nloop > 1:
                with tc.For_i(0, nloop, 1):
                    for wave in range(NW):
                        wave_body(wave)
            else:
                for wave in range(NW):
                    wave_body(wave)

    nc.compile()
    return nc


def _split(x):
    x = np.asarray(x, F32)
    hi = x.astype(BF16)
    lo = (x - hi.astype(F32)).astype(BF16)
    return hi, lo


def prep_inputs(msg, M, input_mask, lengths, emb, Wq, bq, Wc, bc,
                W_ih, W_hh, b_ih, b_hh):
    msg = np.asarray(msg)
    M = np.asarray(M, dtype=F32)
    input_mask = np.asarray(input_mask)
    lengths = np.asarray(lengths)
    emb = np.asarray(emb, dtype=F32)

    # gate reorder [i, f, g, o] -> [i, f, o, g]
    perm = np.concatenate([np.arange(0, H), np.arange(H, 2 * H),
                           np.arange(3 * H, 4 * H), np.arange(2 * H, 3 * H)])
    W_ih_r = np.asarray(W_ih, dtype=F32)[perm]
    W_hh_r = np.asarray(W_hh, dtype=F32)[perm]
    # fold bc into gate bias, bq into score bias
    bg = (np.asarray(b_ih, dtype=F32) + np.asarray(b_hh, dtype=F32)
          + np.asarray(W_hh, dtype=F32) @ np.asarray(bc, dtype=F32))[perm]

    wq_h = np.ascontiguousarray(
        np.asarray(Wq, dtype=F32).T.reshape(KH, 128, E).transpose(1, 0, 2))
    wc_h = np.ascontiguousarray(
        np.asarray(Wc, dtype=F32).T.reshape(KF, 128, H).transpose(1, 0, 2))
    wih_h = np.ascontiguousarray(
        W_ih_r.T.reshape(KE, 128, G4).transpose(1, 0, 2))
    whh_h = np.ascontiguousarray(
        W_hh_r.T.reshape(KH, 128, G4).transpose(1, 0, 2))
    bg_b = np.ascontiguousarray(np.broadcast_to(bg, (BCW, G4))).astype(BF16)
    score_bias = M @ np.asarray(bq, dtype=F32)  # (BS, L)

    xe = emb[msg]  # (BS, T, E) fp32
    lengths_c = np.clip(lengths, 0, T).astype(np.int64)
    mask_bias = (np.where(input_mask, F32(-1e30), F32(0.0)) +
                 score_bias).astype(F32)
    sel = np.zeros((BS, T + 1), dtype=F32)
    sel[np.arange(BS), lengths_c] = 1.0

    wq_hi, wq_lo = _split(wq_h)
    wc_hi, wc_lo = _split(wc_h)
    wih_hi, wih_lo = _split(wih_h)
    whh_hi, whh_lo = _split(whh_h)

    in_maps = []
    for c in range(NCORES):
        bsl = slice(c * BC, (c + 1) * BC)
        Mc = M[bsl]          # (BC, L, E)
        xc = xe[bsl]         # (BC, T, E)
        # wave-major: (NW, BCW, ...)
        m_l = Mc.transpose(1, 0, 2).reshape(L, NW, BCW, E) \
                .transpose(1, 0, 2, 3)                       # (NW,128L,BCW,E)
        m_e = Mc.reshape(NW, BCW, L, KE, 128).transpose(4, 0, 1, 3, 2) \
                .transpose(1, 0, 2, 3, 4)                    # (NW,128e,BCW,KE,L)
        x_c = xc.transpose(2, 1, 0).reshape(KE, 128, T, NW, BCW) \
                .transpose(3, 1, 0, 2, 4)                    # (NW,128e,KE,T,BCW)
        ml_hi, ml_lo = _split(np.ascontiguousarray(m_l))
        me_hi, me_lo = _split(np.ascontiguousarray(m_e))
        x_hi, x_lo = _split(np.ascontiguousarray(x_c))
        in_maps.append({
            "me_hi": me_hi, "me_lo": me_lo, "ml_hi": ml_hi, "ml_lo": ml_lo,
            "x_hi": x_hi, "x_lo": x_lo,
            "maskb": np.ascontiguousarray(mask_bias[bsl].reshape(NW, BCW, L)),
            "sel": np.ascontiguousarray(sel[bsl].reshape(NW, BCW, T + 1)),
            "wq_hi": wq_hi, "wq_lo": wq_lo, "wc_hi": wc_hi, "wc_lo": wc_lo,
            "wih_hi": wih_hi, "wih_lo": wih_lo,
            "whh_hi": whh_hi, "whh_lo": whh_lo,
            "bg": bg_b,
        })
    return in_maps


def kernel(**inputs):
    nloop = int(os.environ.get("KERNEL_NLOOP", "1"))
    if nloop not in _CACHED_NC:
        _CACHED_NC[nloop] = build_nc(nloop)
    nc = _CACHED_NC[nloop]
    in_maps = prep_inputs(**inputs)
    res = bass_utils.run_bass_kernel_spmd(
        nc, in_maps, core_ids=list(range(NCORES)))
    kernel.last_results = res
    out = np.concatenate(
        [np.asarray(r["out"]).reshape(BC, L) for r in res.results], axis=0)
    zeros = np.zeros((BS,), dtype=F32)
    return (out.astype(F32), zeros, zeros.copy())


if __name__ == "__main__":
    nc = build_nc(int(os.environ.get("KERNEL_NLOOP", "1")))
    print("built ok")


# revision 36
# speedup vs baseline: 1.0042x; 1.0042x over previous
"""Trainium2 Bass kernel for nn_AttentionReceiver.

Precision: the 21-step recurrence amplifies perturbations ~100x, so every
matmul runs in software bf16x2 (hi+lo bf16 split, fp32 PSUM accumulate):
products hi*hi + hi*lo + lo*hi(+lo*lo) give ~16-bit mantissa, final rel
error ~3e-4. Softmax/LSTM elementwise in fp32.

Memory: M in both orientations at 4B/elem exceeds SBUF, so each core
processes its 128 batches as two sequential waves of 64. W_ih/W_hh are
re-streamed from HBM per step (hidden under compute).

Layout: batch-on-partition for softmax/LSTM; feature-major (feature on
partition, batch on free) for matmul chains. Per-batch attention matvecs
keep M[b] stationary (128-col bf16 loads use fast-weight-load) and emit
full-partition feature-major columns (base partition 0: always legal).
Steps t=1..19 run in a hardware For_i loop; t=0 (h=0 shortcut: softmax of
the mask alone) and t=20 (attend-only) are peeled.
"""

import os
import sys

for _p in ("/opt/trn_rl_repo", "/opt/trn_rl_repo/concourse"):
    if _p not in sys.path:
        sys.path.insert(0, _p)

import numpy as np
import ml_dtypes

import concourse.bass as bass
import concourse.tile as tile
from concourse import bacc, mybir
from concourse.bass import ds
from concourse.masks import make_identity
from concourse import bass_utils

BF16 = ml_dtypes.bfloat16
F32 = np.float32

BS, L, E, H, T, V = 1024, 128, 256, 512, 20, 1000
NCORES = 8
BC = BS // NCORES    # 128 batches per core
NW = 2               # waves per core
BCW = BC // NW       # 64 batches per wave
KE = E // 128        # 2
KH = H // 128        # 4
KF = (E + H) // 128  # 6
G4 = 4 * H           # 2048

_CACHED_NC = {}


def build_nc(nloop=1, unroll=False):
    fp32 = mybir.dt.float32
    bf16 = mybir.dt.bfloat16

    nc = bacc.Bacc("TRN2", target_bir_lowering=False, debug=False)

    def din(name, shape, dt=bf16):
        return nc.dram_tensor(name, shape, dt, kind="ExternalInput").ap()

    d_me = [din(f"me_{p}", [NW, 128, BCW, KE, 128]) for p in ("hi", "lo")]
    d_ml = [din(f"ml_{p}", [NW, 128, BCW, E]) for p in ("hi", "lo")]
    d_x = [din(f"x_{p}", [NW, 128, KE, T, BCW]) for p in ("hi", "lo")]
    d_mask = din("maskb", [NW, BCW, L], fp32)
    d_sel = din("sel", [NW, BCW, T + 1], fp32)
    d_wq = [din(f"wq_{p}", [128, KH, E]) for p in ("hi", "lo")]
    d_wc = [din(f"wc_{p}", [128, KF, H]) for p in ("hi", "lo")]
    d_wih = [din(f"wih_{p}", [128, KE, G4]) for p in ("hi", "lo")]
    d_whh = [din(f"whh_{p}", [128, KH, G4]) for p in ("hi", "lo")]
    d_bg = din("bg", [BCW, G4], bf16)
    d_out = nc.dram_tensor("out", [NW, BCW, L], fp32, kind="ExternalOutput").ap()

    ADD = mybir.AluOpType.add
    SUB = mybir.AluOpType.subtract
    MULT = mybir.AluOpType.mult
    MAX = mybir.AluOpType.max
    AXX = mybir.AxisListType.X
    EXP = mybir.ActivationFunctionType.Exp
    SIG = mybir.ActivationFunctionType.Sigmoid
    TANH = mybir.ActivationFunctionType.Tanh
    ET = mybir.EngineType

    with tile.TileContext(nc) as tc:
        with (
            tc.tile_pool(name="big", bufs=1) as big,
            tc.tile_pool(name="work", bufs=2) as work,
            tc.tile_pool(name="lstm", bufs=1) as lsp,
            tc.tile_pool(name="xin", bufs=2) as xin,
            tc.tile_pool(name="wst", bufs=2) as wst,
            tc.tile_pool(name="ps_mix", bufs=2, space="PSUM") as ps_mix,
            tc.tile_pool(name="ps_mix2", bufs=2, space="PSUM") as ps_mix2,
            tc.tile_pool(name="ps_g", bufs=2, space="PSUM") as ps_gp,
        ):
            # ---------- resident tiles ----------
            sb_me = [big.tile([128, BCW, KE, 128], bf16, name=f"sb_me_{p}")
                     for p in ("hi", "lo")]
            sb_ml = [big.tile([128, BCW, E], bf16, name=f"sb_ml_{p}")
                     for p in ("hi", "lo")]
            sb_wq = [big.tile([128, KH, E], bf16, name=f"sb_wq_{p}")
                     for p in ("hi", "lo")]
            sb_wc = [big.tile([128, KF, H], bf16, name=f"sb_wc_{p}")
                     for p in ("hi", "lo")]
            sb_bg = big.tile([BCW, G4], bf16)
            sb_mask = big.tile([BCW, L], fp32)
            sb_sel = big.tile([BCW, T + 1], fp32)
            ident32 = big.tile([128, 128], fp32)
            identb = big.tile([128, 128], bf16)
            h_fm = [big.tile([128, KH, BCW], bf16, name=f"h_fm_{p}")
                    for p in ("hi", "lo")]
            c_bm = big.tile([BCW, H], fp32)
            out_sb = big.tile([BCW, L], fp32)

            for p in range(2):
                nc.sync.dma_start(out=sb_wq[p], in_=d_wq[p])
                nc.sync.dma_start(out=sb_wc[p], in_=d_wc[p])
            nc.sync.dma_start(out=sb_bg, in_=d_bg)
            make_identity(nc, ident32)
            make_identity(nc, identb)

            def mm(out, lhsT, rhs, start, stop):
                nc.tensor.matmul(out, lhsT=lhsT, rhs=rhs, start=start,
                                 stop=stop)

            def softmax_accum(sc_src, t_sel):
                """softmax over L + sel-weighted fp32 accumulation.
                sc_src: [BCW, L] fp32 AP (already mask-included).
                Returns w_f32 [BCW, L]."""
                mx = work.tile([BCW, 1], fp32, tag="mx")
                nc.vector.tensor_reduce(mx, sc_src, axis=AXX, op=MAX)
                nmx = work.tile([BCW, 1], fp32, tag="nmx")
                nc.vector.tensor_scalar_mul(out=nmx, in0=mx, scalar1=-1.0)
                w_exp = work.tile([BCW, L], fp32, tag="w_exp")
                sume = work.tile([BCW, 1], fp32, tag="sume")
                nc.scalar.activation(w_exp, sc_src, EXP, bias=nmx, scale=1.0,
                                     accum_out=sume)
                rsum = work.tile([BCW, 1], fp32, tag="rsum")
                nc.vector.reciprocal(rsum, sume)
                w_f32 = work.tile([BCW, L], fp32, tag="w_f32")
                nc.vector.tensor_tensor(
                    w_f32, w_exp, rsum[:, 0:1].to_broadcast((BCW, L)), MULT)
                rs2 = work.tile([BCW, 1], fp32, tag="rs2")
                nc.vector.tensor_tensor(rs2, rsum, t_sel, MULT)
                wtmp = work.tile([BCW, L], fp32, tag="wtmp")
                nc.vector.tensor_tensor(
                    wtmp, w_exp, rs2[:, 0:1].to_broadcast((BCW, L)), MULT)
                nc.vector.tensor_tensor(out_sb, out_sb, wtmp, ADD)
                return w_f32

            def attend_scores(t_sel):
                """pq + scores + softmax + accumulate. Returns w_f32."""
                # pq = Wq @ h   (feature-major [e, b], bf16x2)
                ps_pq = ps_mix.tile([128, KE, BCW], fp32, tag="ps_mix")
                for mt in range(KE):
                    for kt in range(KH):
                        lw_hi = sb_wq[0][:, kt, mt * 128:(mt + 1) * 128]
                        lw_lo = sb_wq[1][:, kt, mt * 128:(mt + 1) * 128]
                        o = ps_pq[:, mt, :]
                        mm(o, lw_hi, h_fm[0][:, kt, :], kt == 0, False)
                        mm(o, lw_hi, h_fm[1][:, kt, :], False, False)
                        mm(o, lw_lo, h_fm[0][:, kt, :], False, kt == KH - 1)
                # split pq -> [hi|lo] adjacent pairs
                pq_hl = work.tile([128, KE, BCW, 2], bf16, tag="pq_hl")
                nc.vector.tensor_copy(pq_hl[:, :, :, 0], ps_pq)
                nc.vector.tensor_tensor(pq_hl[:, :, :, 1], ps_pq,
                                        pq_hl[:, :, :, 0], SUB)
                # scores.T columns: stationary M[b] chunks, rhs = pq pair
                ps_sc = ps_mix2.tile([128, BCW, 2], fp32, tag="ps_mix2")
                for b in range(BCW):
                    o = ps_sc[:, b, :]
                    for ke in range(KE):
                        mm(o, sb_me[0][:, b, ke, :], pq_hl[:, ke, b, :],
                           ke == 0, False)
                        mm(o, sb_me[1][:, b, ke, :], pq_hl[:, ke, b, :],
                           False, ke == KE - 1)
                sc_pair = work.tile([128, BCW, 2], fp32, tag="sc_pair")
                nc.vector.tensor_copy(sc_pair, ps_sc)
                sc_fm = work.tile([128, BCW], fp32, tag="sc_fm")
                nc.vector.tensor_tensor(sc_fm, sc_pair[:, :, 0],
                                        sc_pair[:, :, 1], ADD)
                ps_scB = ps_mix2.tile([BCW, 128], fp32, tag="ps_mix2")
                nc.tensor.transpose(ps_scB, sc_fm, ident32)
                sc_sb = work.tile([BCW, L], fp32, tag="sc_sb")
                nc.vector.tensor_tensor(sc_sb, ps_scB, sb_mask, ADD)
                return softmax_accum(sc_sb, t_sel)

            def applied_att(w_f32):
                """w split/transpose, applied matvecs, att projection.
                Returns (att_hi, att_lo) feature-major [128, KH, BCW]."""
                w_hi = work.tile([BCW, L], bf16, tag="w_hi")
                nc.vector.tensor_copy(w_hi, w_f32)
                w_lo = work.tile([BCW, L], bf16, tag="w_lo")
                nc.vector.tensor_tensor(w_lo, w_f32, w_hi, SUB)
                w_fhl = work.tile([128, BCW, 2], bf16, tag="w_fhl")
                for p, wt in ((0, w_hi), (1, w_lo)):
                    ps_wT = ps_mix2.tile([128, BCW], bf16, tag="ps_mix2")
                    nc.tensor.transpose(ps_wT, wt, identb[0:BCW, 0:BCW])
                    nc.vector.tensor_copy(w_fhl[:, :, p], ps_wT)
                # applied.T columns: stationary M_l[b] chunks, rhs = w pair
                ps_ap = ps_mix.tile([128, KE, BCW, 2], fp32, tag="ps_mix")
                for b in range(BCW):
                    wpair = w_fhl[:, b, :]
                    for mt in range(KE):
                        o = ps_ap[:, mt, b, :]
                        mm(o, sb_ml[0][:, b, mt * 128:(mt + 1) * 128], wpair,
                           True, False)
                        mm(o, sb_ml[1][:, b, mt * 128:(mt + 1) * 128], wpair,
                           False, True)
                ap_pair = work.tile([128, KE, BCW, 2], fp32, tag="ap_pair")
                nc.vector.tensor_copy(ap_pair, ps_ap)
                ap_f32 = work.tile([128, KE, BCW], fp32, tag="ap_f32")
                nc.vector.tensor_tensor(ap_f32, ap_pair[:, :, :, 0],
                                        ap_pair[:, :, :, 1], ADD)
                ap_hi = work.tile([128, KE, BCW], bf16, tag="ap_hi")
                nc.vector.tensor_copy(ap_hi, ap_f32)
                ap_lo = work.tile([128, KE, BCW], bf16, tag="ap_lo")
                nc.vector.tensor_tensor(ap_lo, ap_f32, ap_hi, SUB)
                # att = Wc @ [applied; h]  (feature-major, bf16x2)
                ps_att = ps_mix.tile([128, KH, BCW], fp32, tag="ps_mix")
                for mt in range(KH):
                    o = ps_att[:, mt, :]
                    for kf in range(KF):
                        if kf < KE:
                            r_hi, r_lo = ap_hi[:, kf, :], ap_lo[:, kf, :]
                        else:
                            r_hi = h_fm[0][:, kf - KE, :]
                            r_lo = h_fm[1][:, kf - KE, :]
                        lw_hi = sb_wc[0][:, kf, mt * 128:(mt + 1) * 128]
                        lw_lo = sb_wc[1][:, kf, mt * 128:(mt + 1) * 128]
                        mm(o, lw_hi, r_hi, kf == 0, False)
                        mm(o, lw_hi, r_lo, False, False)
                        mm(o, lw_lo, r_hi, False, kf == KF - 1)
                att_hi = work.tile([128, KH, BCW], bf16, tag="att_hi")
                nc.vector.tensor_copy(att_hi, ps_att)
                att_lo = work.tile([128, KH, BCW], bf16, tag="att_lo")
                nc.vector.tensor_tensor(att_lo, ps_att, att_hi, SUB)
                return att_hi, att_lo

            def lstm_step(wave, t_dyn, att_hi, att_lo):
                """gates (streamed W) + LSTM elementwise + h split/transpose."""
                x_t = []
                for p in range(2):
                    xt = xin.tile([128, KE, 1, BCW], bf16, tag=f"x_{p}")
                    nc.sync.dma_start(out=xt, in_=d_x[p][wave][:, :, t_dyn, :])
                    x_t.append(xt)
                gsum = lsp.tile([BCW, G4], fp32, tag="gsum")
                NCH = 256
                for half in range(2):
                    ps_gh = ps_gp.tile([BCW, 1024], fp32, tag="ps_g")
                    for nch in range(1024 // NCH):
                        n0 = half * 1024 + nch * NCH
                        wb = []
                        for p in range(2):
                            wi = wst.tile([128, KE, NCH], bf16, tag=f"wih_{p}")
                            nc.sync.dma_start(out=wi,
                                              in_=d_wih[p][:, :, n0:n0 + NCH])
                            wh = wst.tile([128, KH, NCH], bf16, tag=f"whh_{p}")
                            nc.sync.dma_start(out=wh,
                                              in_=d_whh[p][:, :, n0:n0 + NCH])
                            wb.append((wi, wh))
                        o = ps_gh[:, nch * NCH:(nch + 1) * NCH]
                        for kt in range(KF):
                            if kt < KE:
                                a_hi = x_t[0][:, kt, 0, :]
                                a_lo = x_t[1][:, kt, 0, :]
                                r_hi = wb[0][0][:, kt, :]
                                r_lo = wb[1][0][:, kt, :]
                            else:
                                a_hi = att_hi[:, kt - KE, :]
                                a_lo = att_lo[:, kt - KE, :]
                                r_hi = wb[0][1][:, kt - KE, :]
                                r_lo = wb[1][1][:, kt - KE, :]
                            mm(o, a_hi, r_hi, kt == 0, False)
                            mm(o, a_hi, r_lo, False, False)
                            mm(o, a_lo, r_hi, False, kt == KF - 1)
                    nc.vector.tensor_tensor(
                        gsum[:, half * 1024:(half + 1) * 1024], ps_gh,
                        sb_bg[:, half * 1024:(half + 1) * 1024], ADD)
                # gates order [i, f, o, g]
                sig_if = lsp.tile([BCW, 2 * H], fp32, tag="sig_if")
                nc.scalar.activation(sig_if, gsum[:, 0:2 * H], SIG)
                sig_o = lsp.tile([BCW, H], fp32, tag="sig_o")
                nc.scalar.activation(sig_o, gsum[:, 2 * H:3 * H], SIG)
                tanh_g = lsp.tile([BCW, H], fp32, tag="tanh_g")
                nc.scalar.activation(tanh_g, gsum[:, 3 * H:4 * H], TANH)
                t1 = lsp.tile([BCW, H], fp32, tag="t1")
                nc.vector.tensor_tensor(t1, sig_if[:, 0:H], tanh_g, MULT)
                nc.vector.tensor_tensor(c_bm, c_bm, sig_if[:, H:2 * H], MULT)
                nc.vector.tensor_tensor(c_bm, c_bm, t1, ADD)
                tanh_c = lsp.tile([BCW, H], fp32, tag="tanh_c")
                nc.scalar.activation(tanh_c, c_bm, TANH)
                h_f32 = lsp.tile([BCW, H], fp32, tag="h_f32")
                nc.vector.tensor_tensor(h_f32, sig_o, tanh_c, MULT)
                h_hi = lsp.tile([BCW, H], bf16, tag="h_hi")
                nc.vector.tensor_copy(h_hi, h_f32)
                h_lo = lsp.tile([BCW, H], bf16, tag="h_lo")
                nc.vector.tensor_tensor(h_lo, h_f32, h_hi, SUB)
                for p, hb in ((0, h_hi), (1, h_lo)):
                    for kt in range(KH):
                        ps_h = ps_mix2.tile([128, BCW], bf16, tag="ps_mix2")
                        nc.tensor.transpose(
                            ps_h, hb[:, kt * 128:(kt + 1) * 128],
                            identb[0:BCW, 0:BCW])
                        nc.vector.tensor_copy(h_fm[p][:, kt, :], ps_h)

            def wave_body(wave):
                # M + per-wave small tensors
                for p in range(2):
                    for q in range(4):
                        sl = slice(q * 16, (q + 1) * 16)
                        nc.sync.dma_start(out=sb_ml[p][:, sl],
                                          in_=d_ml[p][wave][:, sl])
                    for q in range(4):
                        sl = slice(q * 16, (q + 1) * 16)
                        nc.sync.dma_start(out=sb_me[p][:, sl],
                                          in_=d_me[p][wave][:, sl])
                nc.sync.dma_start(out=sb_mask, in_=d_mask[wave])
                nc.sync.dma_start(out=sb_sel, in_=d_sel[wave])
                nc.vector.memset(h_fm[0], 0.0)
                nc.vector.memset(h_fm[1], 0.0)
                nc.vector.memset(c_bm, 0.0)
                nc.vector.memset(out_sb, 0.0)

                # t=0 peeled: h=0 -> scores = mask only
                w0 = softmax_accum(sb_mask, sb_sel[:, 0:1])
                a_hi, a_lo = applied_att(w0)
                lstm_step(wave, ds(0, 1), a_hi, a_lo)
                # t = 1..19
                def mid_step(t):
                    wf = attend_scores(sb_sel[:, ds(t, 1)])
                    a_hi, a_lo = applied_att(wf)
                    lstm_step(wave, ds(t, 1), a_hi, a_lo)

                if unroll:
                    for t in range(1, T):
                        mid_step(t)
                else:
                    # unroll-by-2: barrier every 2 steps, lets Tile overlap
                    # step t+1's x-gates under step t's LSTM tail
                    with tc.For_i(1, T - 1, 2,
                                  hint_engines=(ET.PE, ET.DVE, ET.Activation,
                                                ET.SP)) as t:
                        mid_step(t)
                        mid_step(t + 1)
                    mid_step(T - 1)
                # t=20 peeled: attend only
                attend_scores(sb_sel[:, T:T + 1])
                nc.sync.dma_start(out=d_out[wave], in_=out_sb)

            if nloop > 1:
                with tc.For_i(0, nloop, 1):
                    for wave in range(NW):
                        wave_body(wave)
            else:
                for wave in range(NW):
                    wave_body(wave)

    nc.compile()
    return nc


def _split(x):
    x = np.asarray(x, F32)
    hi = x.astype(BF16)
    lo = (x - hi.astype(F32)).astype(BF16)
    return hi, lo


def prep_inputs(msg, M, input_mask, lengths, emb, Wq, bq, Wc, bc,
                W_ih, W_hh, b_ih, b_hh):
    msg = np.asarray(msg)
    M = np.asarray(M, dtype=F32)
    input_mask = np.asarray(input_mask)
    lengths = np.asarray(lengths)
    emb = np.asarray(emb, dtype=F32)

    # gate reorder [i, f, g, o] -> [i, f, o, g]
    perm = np.concatenate([np.arange(0, H), np.arange(H, 2 * H),
                           np.arange(3 * H, 4 * H), np.arange(2 * H, 3 * H)])
    W_ih_r = np.asarray(W_ih, dtype=F32)[perm]
    W_hh_r = np.asarray(W_hh, dtype=F32)[perm]
    # fold bc into gate bias, bq into score bias
    bg = (np.asarray(b_ih, dtype=F32) + np.asarray(b_hh, dtype=F32)
          + np.asarray(W_hh, dtype=F32) @ np.asarray(bc, dtype=F32))[perm]

    wq_h = np.ascontiguousarray(
        np.asarray(Wq, dtype=F32).T.reshape(KH, 128, E).transpose(1, 0, 2))
    wc_h = np.ascontiguousarray(
        np.asarray(Wc, dtype=F32).T.reshape(KF, 128, H).transpose(1, 0, 2))
    wih_h = np.ascontiguousarray(
        W_ih_r.T.reshape(KE, 128, G4).transpose(1, 0, 2))
    whh_h = np.ascontiguousarray(
        W_hh_r.T.reshape(KH, 128, G4).transpose(1, 0, 2))
    bg_b = np.ascontiguousarray(np.broadcast_to(bg, (BCW, G4))).astype(BF16)
    score_bias = M @ np.asarray(bq, dtype=F32)  # (BS, L)

    xe = emb[msg]  # (BS, T, E) fp32
    lengths_c = np.clip(lengths, 0, T).astype(np.int64)
    mask_bias = (np.where(input_mask, F32(-1e30), F32(0.0)) +
                 score_bias).astype(F32)
    sel = np.zeros((BS, T + 1), dtype=F32)
    sel[np.arange(BS), lengths_c] = 1.0

    wq_hi, wq_lo = _split(wq_h)
    wc_hi, wc_lo = _split(wc_h)
    wih_hi, wih_lo = _split(wih_h)
    whh_hi, whh_lo = _split(whh_h)

    in_maps = []
    for c in range(NCORES):
        bsl = slice(c * BC, (c + 1) * BC)
        Mc = M[bsl]          # (BC, L, E)
        xc = xe[bsl]         # (BC, T, E)
        # wave-major: (NW, BCW, ...)
        m_l = Mc.transpose(1, 0, 2).reshape(L, NW, BCW, E) \
                .transpose(1, 0, 2, 3)                       # (NW,128L,BCW,E)
        m_e = Mc.reshape(NW, BCW, L, KE, 128).transpose(4, 0, 1, 3, 2) \
                .transpose(1, 0, 2, 3, 4)                    # (NW,128e,BCW,KE,L)
        x_c = xc.transpose(2, 1, 0).reshape(KE, 128, T, NW, BCW) \
                .transpose(3, 1, 0, 2, 4)                    # (NW,128e,KE,T,BCW)
        ml_hi, ml_lo = _split(np.ascontiguousarray(m_l))
        me_hi, me_lo = _split(np.ascontiguousarray(m_e))
        x_hi, x_lo = _split(np.ascontiguousarray(x_c))
        in_maps.append({
            "me_hi": me_hi, "me_lo": me_lo, "ml_hi": ml_hi, "ml_lo": ml_lo,
            "x_hi": x_hi, "x_lo": x_lo,
            "maskb": np.ascontiguousarray(mask_bias[bsl].reshape(NW, BCW, L)),
            "sel": np.ascontiguousarray(sel[bsl].reshape(NW, BCW, T + 1)),
            "wq_hi": wq_hi, "wq_lo": wq_lo, "wc_hi": wc_hi, "wc_lo": wc_lo,
            "wih_hi": wih_hi, "wih_lo": wih_lo,
            "whh_hi": whh_hi, "whh_lo": whh_lo,
            "bg": bg_b,
        })
    return in_maps


def kernel(**inputs):
    nloop = int(os.environ.get("KERNEL_NLOOP", "1"))
    if nloop not in _CACHED_NC:
        _CACHED_NC[nloop] = build_nc(nloop)
    nc = _CACHED_NC[nloop]
    in_maps = prep_inputs(**inputs)
    res = bass_utils.run_bass_kernel_spmd(
        nc, in_maps, core_ids=list(range(NCORES)))
    kernel.last_results = res
    out = np.concatenate(
        [np.asarray(r["out"]).reshape(BC, L) for r in res.results], axis=0)
    zeros = np.zeros((BS,), dtype=F32)
    return (out.astype(F32), zeros, zeros.copy())


if __name__ == "__main__":
    nc = build_nc(int(os.environ.get("KERNEL_NLOOP", "1")))
    print("built ok")
